# revision 19
# baseline (speedup 1.0000x reference)
"""Trainium2 Bass kernel for nn_ConvAttention: LayerNorm -> 1x1-conv QKV ->
per-(b,h)-row attention over W -> skip connection.

Sharding: data-parallel over batch B=8 across 8 NeuronCores. Each core
processes 64 (h) slabs of [W=256, C=256].

Numerics strategy: fp16 everywhere on the input side. fp16 has the same
10-bit mantissa as float32r (TF32), so casting x/W to fp16 on the host
loses nothing vs the f32r matmul path -- and it halves the input DMA
traffic, doubles/quadruples DVE throughput on the 2-byte elementwise ops
(2x/4x DVE perf modes), and runs PE transposes at 1.0 cycles/row instead
of f32r's 1.5. Matmuls accumulate in f32 PSUM as always. E (softmax
numerator) stays bf16 because exp(s - SHIFT) can exceed fp16's 65504
range.

Engine balance per slab (Pool/gpsimd cannot read PSUM on TRN2 and its
software elementwise path is ~15ns/elem, so all PSUM evictions live on
DVE/ACT, batched into as few instructions as possible):
  PE:   4 transposes + 4 qk + 2 sT + 4 v + 4 y matmuls
  DVE:  bn_stats/aggr, LN apply (4x mode), pair-batched xnT evict (2x),
        recip, y-normalize half + fp16 skip-adds
  ACT:  4-slab-batched rsqrt chain (Ln+Exp), merged qk+v evict,
        pair-batched softmax exp, y-normalize half
Softmax max-subtraction is replaced by a constant shift (exact in real
arithmetic); scores are computed transposed (s^T = k @ q^T) so the exp
output feeds the y-matmul directly as lhsT. Z comes from ones columns
appended to the V operand.
"""

import os
import sys

for _p in ("/opt/trn_rl_repo", "/root/.axon_site/_ro/trn_rl_repo"):
    if _p not in sys.path:
        sys.path.insert(0, _p)

import numpy as np

import concourse.tile as tile
from concourse import bacc, mybir
from concourse.bass_utils import run_bass_kernel_spmd
from concourse.masks import make_identity

F32 = mybir.dt.float32
F16 = mybir.dt.float16
BF16 = mybir.dt.bfloat16
AF = mybir.ActivationFunctionType
ALU = mybir.AluOpType

B, H, W, C = 8, 64, 256, 256
F2 = 2 * C
NS = H  # slabs per core (batch-sharded over 8 cores)
EPS = 1e-3  # Keras LayerNormalization default
SHIFT = 32.0  # constant softmax shift (replaces per-row max subtraction)

_NC_CACHE: dict = {}


def _install_act_root():
    """Reorder act_info.json so natural_log_exp_and_others is the first set:
    bass' first-match table chooser then resolves Ln, Exp, Identity and Copy
    to one set, avoiding per-slab ACT table reloads (~2.7us each)."""
    if os.environ.get("BASS_ACT_ROOT_JSON_PATH"):
        return
    try:
        import json
        import tempfile

        import neuronxcc.driver.jobs.support.FindActInfo as FAI
        from neuronxcc.driver.Job import Job

        src = FAI.findActInfoFile(Job.getPackageDir(), "gen3")
        srcdir = os.path.dirname(src)
        d = json.load(open(src))
        sets = d["act_func_sets"]
        first = [s for s in sets if s["name"] == "natural_log_exp_and_others"]
        if not first:
            return
        rest = [s for s in sets if s["name"] != "natural_log_exp_and_others"]
        d["act_func_sets"] = first + rest
        td = tempfile.mkdtemp(prefix="act_root_")
        for fn in os.listdir(srcdir):
            sp = os.path.join(srcdir, fn)
            if os.path.isfile(sp) and fn != os.path.basename(src):
                os.symlink(sp, os.path.join(td, fn))
        out = os.path.join(td, os.path.basename(src))
        with open(out, "w") as f:
            json.dump(d, f)
        os.environ["BASS_ACT_ROOT_JSON_PATH"] = out
        _orig = FAI.findActInfoFile
        FAI.findActInfoFile = lambda *a, **k: out
        import concourse.hw_specs as hw_specs

        hw_specs.get_activation_tables.cache_clear()
    except Exception as e:  # noqa: BLE001
        print(f"act root override failed (table thrash will persist): {e}")


def _build(with_bias: bool):
    _install_act_root()
    nc = bacc.Bacc("TRN2", target_bir_lowering=False, debug=False, num_devices=8)
    # x host-interleaved to [s, p, c, t]: both w-rows of a partition sit
    # even/odd in the free axis, so ONE bn_stats yields both rows' stats
    x_d = nc.dram_tensor("x", [NS, 128, C, 2], F16, kind="ExternalInput").ap()
    # weights in natural lhsT layout [c, f], split into two 128-c chunks
    wqk_d = nc.dram_tensor("wqk", [2, 128, 256], F16, kind="ExternalInput").ap()
    wv_d = nc.dram_tensor("wv", [2, 128, 256], F16, kind="ExternalInput").ap()
    bqk_d = bv_d = None
    if with_bias:
        bqk_d = nc.dram_tensor("bqk", [2, 128], F32, kind="ExternalInput").ap()
        bv_d = nc.dram_tensor("bv", [256], F32, kind="ExternalInput").ap()
    out_d = nc.dram_tensor("out", [NS, W, C], F16, kind="ExternalOutput").ap()

    # per-slab views
    x_r = x_d
    out_r = out_d.rearrange("s (t p) c -> s p t c", p=128)

    with tile.TileContext(nc) as tc:
        _emit(nc, tc, x_r, out_r, wqk_d, wv_d, bqk_d, bv_d)
    nc.compile()
    return nc


def _emit(nc, tc, x_r, out_r, wqk_d, wv_d, bqk_d, bv_d):
    from contextlib import ExitStack

    with ExitStack() as ctx:
        ec = ctx.enter_context
        consts = ec(tc.tile_pool(name="consts", bufs=1))
        xpool = ec(tc.tile_pool(name="xp", bufs=20))
        xnpool = ec(tc.tile_pool(name="xnp", bufs=4))
        xtpool = ec(tc.tile_pool(name="xtp", bufs=4))
        qvpool = ec(tc.tile_pool(name="qvp", bufs=6))
        epool = ec(tc.tile_pool(name="ep", bufs=3))
        opool = ec(tc.tile_pool(name="op", bufs=6))
        ypool = ec(tc.tile_pool(name="yp", bufs=3))
        stat = ec(tc.tile_pool(name="stat", bufs=8))
        # PSUM banks (8x2KB): xnT pair-tile 1 bank, qv 2 banks, sT
        # pair-tile 2 banks, y three 1-bank tiles (1.5-slab double
        # buffering of the tail) -> 8 total
        ps_xnT = ec(tc.tile_pool(name="ps_xnT", bufs=1, space="PSUM"))
        ps_qv = ec(tc.tile_pool(name="ps_qv", bufs=1, space="PSUM"))
        ps_sT = ec(tc.tile_pool(name="ps_sT", bufs=1, space="PSUM"))
        ps_y0 = ec(tc.tile_pool(name="ps_y0", bufs=2, space="PSUM"))
        ps_y1 = ec(tc.tile_pool(name="ps_y1", bufs=1, space="PSUM"))

        ident = consts.tile([128, 128], F16)
        make_identity(nc, ident)
        negshift = consts.tile([128, 1], F32)
        nc.vector.memset(negshift, -SHIFT)
        eps_t = consts.tile([128, 1], F32)
        nc.vector.memset(eps_t, EPS)

        wqk = consts.tile([128, 2, 256], F16)
        nc.sync.dma_start(wqk, wqk_d.rearrange("t p f -> p t f"))
        wv = consts.tile([128, 2, 256], F16)
        nc.sync.dma_start(wv, wv_d.rearrange("t p f -> p t f"))

        if bqk_d is not None:
            import concourse.bass as bass
            bqk_sb = consts.tile([128, 2], F32)
            nc.sync.dma_start(bqk_sb, bqk_d.rearrange("t p -> p t"))
            bvf = consts.tile([128, 2, 256], F32)
            bv_b = bass.AP(tensor=bv_d.tensor, offset=bv_d.offset,
                           ap=[[0, 128], [0, 2], [1, 256]])
            nc.sync.dma_start(bvf, bv_b)

        xdma = {}

        def prefetch_x(s0, n=4):
            for si in range(n):
                if s0 + si < NS:
                    x_sb = xpool.tile([128, 256, 2], F16)
                    nc.sync.dma_start(x_sb, x_r[s0 + si])
                    xdma[s0 + si] = x_sb

        grp = {}

        def stats_slab(s):
            """Per-slab LN stats, emitted 8 slabs ahead of use so the DVE
            load is spread evenly (a burst here queues ahead of the
            latency-critical xnT evict and starves the PE)."""
            if s % 4 == 0:
                prefetch_x(s + 6, 4)
                st4 = stat.tile([128, 4, 6], F32, name="st4")
                g = {"st4": st4,
                     "st4r": st4.rearrange("p g (a b) -> p g a b", b=3)}
                grp[s // 4] = g
            g = grp[s // 4]
            si = s % 4
            x_sb = xdma.pop(s)
            nc.vector.bn_stats(g["st4"][:, si, :],
                               x_sb.rearrange("p c t -> p (c t)"))
            st[s] = {"x_sb": x_sb, "st4r": g["st4r"], "si": si, "s": s}
            if si == 3 or s == NS - 1:
                # rs = rsqrt(var+eps) = exp(-0.5*ln(var+eps)); ln+exp live
                # in one ACT table set (see _install_act_root)
                g = grp.pop(s // 4)
                lnv4 = stat.tile([128, 4, 2, 1], F32)
                # var = M2/256: fold the 1/256 into Ln's input scale
                nc.scalar.activation(out=lnv4, in_=g["st4r"][:, :, :, 2:3],
                                     func=AF.Ln, bias=eps_t, scale=1.0 / 256)
                rs4 = stat.tile([128, 4, 2, 1], F32)
                nc.scalar.activation(out=rs4, in_=lnv4, func=AF.Exp,
                                     scale=-0.5)
                for si2 in range(4):
                    s2 = (s // 4) * 4 + si2
                    if s2 < NS:
                        st[s2]["rs4"] = rs4

        def front_xn(s0):
            """Fused LN apply xn = (x - mu) * rs on DVE (4x perf mode: all
            fp16 SBUF operands), for slabs s0, s0+1."""
            for si in (0, 1):
                d = st[s0 + si]
                xn = xnpool.tile([128, 2, 256], F16)
                for t in (0, 1):
                    nc.vector.tensor_scalar(
                        out=xn[:, t, :], in0=d["x_sb"][:, :, t],
                        scalar1=d["st4r"][:, d["si"], t, 1:2],
                        scalar2=d["rs4"][:, d["si"], t, :],
                        op0=ALU.subtract, op1=ALU.mult)
                d["xn"] = xn
                d.pop("st4r"), d.pop("rs4")

        def emit_transpose(d):
            """Transpose xn -> [c, w] (PE, fp16 1.0 cyc/row) into the pair's
            shared PSUM bank; DVE-evict once per pair (2x fp16 mode)."""
            s = d["s"]
            if s % 2 == 0:
                d["p_xnT2"] = ps_xnT.tile([128, 2, 2, 256], F16, name="p_xnT2")
            p2 = st[s - 1]["p_xnT2"] if s % 2 else d["p_xnT2"]
            if s % 2:
                d["p_xnT2"] = p2
            xn = d.pop("xn")
            for cc in (0, 1):
                for t in (0, 1):
                    nc.tensor.transpose(
                        p2[:, s % 2, cc, t * 128:(t + 1) * 128],
                        xn[:, t, cc * 128:(cc + 1) * 128], ident)
            if s % 2 == 1:
                xnT2 = xtpool.tile([128, 2, 2, 256], F16)
                nc.vector.tensor_copy(xnT2, p2)  # fp16 2x DVE mode
                st[s - 1]["xnT2"], st[s - 1]["sj"] = xnT2, 0
                d["xnT2"], d["sj"] = xnT2, 1
                st[s - 1].pop("p_xnT2"), d.pop("p_xnT2")

        def emit_qkv(d):
            """qk^T and v matmuls into one 2-bank PSUM tile; single merged
            ACT eviction -> fp16 SBUF [qk | v | ones]."""
            xnT = d.pop("xnT2")
            sj = d.pop("sj")
            p_qv = ps_qv.tile([128, 2, 512], F32)
            for blk in (0, 1):
                for cc in (0, 1):
                    nc.tensor.matmul(
                        p_qv[:, blk, 0:256],
                        wqk[:, cc, blk * 128:(blk + 1) * 128],
                        xnT[:, sj, cc, :],
                        start=(cc == 0), stop=(cc == 1))
            for jt in (0, 1):
                for cc in (0, 1):
                    nc.tensor.matmul(
                        p_qv[:, jt, 256:512],
                        xnT[:, sj, cc, jt * 128:(jt + 1) * 128],
                        wv[:, cc, :],
                        start=(cc == 0), stop=(cc == 1))
            if bqk_d is not None:
                for blk in (0, 1):
                    nc.vector.tensor_scalar(
                        out=p_qv[:, blk, 0:256], in0=p_qv[:, blk, 0:256],
                        scalar1=bqk_sb[:, blk:blk + 1], scalar2=None,
                        op0=ALU.add)
            qv = qvpool.tile([128, 2, 514], F16)
            nc.scalar.copy(qv[:, :, 0:512], p_qv)
            # ones columns accumulate Z in the y-matmul
            nc.gpsimd.memset(qv[:, :, 512:514], 1.0)
            d["qv"] = qv

        def emit_scores(d):
            """s^T matmuls into the pair's shared PSUM tile; exp once per
            pair -> E (bf16 softmax weights)."""
            s, qv = d["s"], d["qv"]
            if s % 2 == 0:
                d["p_sT2"] = ps_sT.tile([128, 2, 2, 256], F32, name="p_sT2")
            p2 = st[s - 1]["p_sT2"] if s % 2 else d["p_sT2"]
            if s % 2:
                d["p_sT2"] = p2
            for jt in (0, 1):
                nc.tensor.matmul(
                    p2[:, s % 2, jt, :],
                    qv[:, 1, jt * 128:(jt + 1) * 128],
                    qv[:, 0, 0:256],
                    start=True, stop=True)
            if s % 2 == 1:
                E2 = epool.tile([128, 2, 2, 256], BF16)
                nc.scalar.activation(out=E2, in_=p2, func=AF.Exp,
                                     bias=negshift, scale=1.0)
                st[s - 1]["E2"], st[s - 1]["ei"] = E2, 0
                d["E2"], d["ei"] = E2, 1
                st[s - 1].pop("p_sT2"), d.pop("p_sT2")

        def emit_tail_a(d):
            """y-matmul, 1/Z normalize, half the skip-add. The
            normalize+add is split: it=0 fused stt on DVE, it=1 ACT scale
            now + DVE fp16 add next iteration (so DVE never waits on ACT)."""
            E2, ei, qv, x_sb = d["E2"], d["ei"], d.pop("qv"), d["x_sb"]
            p_y0 = ps_y0.tile([128, 258], F32, name="p_y0")
            p_y1 = ps_y1.tile([128, 258], F32, name="p_y1")
            for it, p_y in ((0, p_y0), (1, p_y1)):
                for jt in (0, 1):
                    nc.tensor.matmul(
                        p_y[:, 0:258],
                        E2[:, ei, jt, it * 128:(it + 1) * 128],
                        qv[:, jt, 256:514],
                        start=(jt == 0), stop=(jt == 1))
            rZ = stat.tile([128, 2, 1], F32)
            nc.vector.reciprocal(rZ[:, 0, :], p_y0[:, 256:257])
            nc.vector.reciprocal(rZ[:, 1, :], p_y1[:, 256:257])
            o_sb = opool.tile([128, 2, 256], F16)
            # it=0: out = x + y*rZ fused on DVE
            nc.vector.scalar_tensor_tensor(
                out=o_sb[:, 0, :], in0=p_y0[:, 0:256],
                scalar=rZ[:, 0, :], in1=x_sb[:, :, 0],
                op0=ALU.mult, op1=ALU.add)
            # it=1: ACT evicts y*rZ to fp16 ...
            yt = ypool.tile([128, 256], F16)
            nc.scalar.activation(out=yt, in_=p_y1[:, 0:256], func=AF.Copy,
                                 scale=rZ[:, 1, :])
            d["o_sb"], d["yt"] = o_sb, yt

        def emit_tail_b(d):
            """... DVE adds x (2x fp16 mode) one iteration later, store."""
            o_sb, yt, x_sb, s = d["o_sb"], d["yt"], d["x_sb"], d["s"]
            nc.vector.tensor_tensor(out=o_sb[:, 1, :], in0=yt,
                                    in1=x_sb[:, :, 1], op=ALU.add)
            if bv_d is not None:
                nc.gpsimd.tensor_tensor(out=o_sb, in0=o_sb, in1=bvf,
                                        op=ALU.add)
            nc.sync.dma_start(out_r[s], o_sb)

        # software pipeline, slab s: stats@<=s, xn@s|s+1, transpose@s+1,
        # xnT-evict@pair end, qkv@s+2, scores@s+3, exp@pair end,
        # tail_a@s+5, tail_b@s+6
        st = {}
        prefetch_x(0, 14)
        for s in range(8):
            stats_slab(s)
        for i in range(NS + 7):
            if i + 8 < NS:
                stats_slab(i + 8)
            if i - 6 >= 0 and i - 6 < NS:
                emit_tail_a(st[i - 6])
            if 0 <= i - 1 < NS:
                emit_transpose(st[i - 1])
            if i - 7 >= 0:
                emit_tail_b(st.pop(i - 7))
            if 0 <= i - 3 < NS:
                emit_qkv(st[i - 3])
            if 0 <= i - 4 < NS:
                emit_scores(st[i - 4])
            if i < NS and i % 2 == 0:
                front_xn(i)


def _install_ntff_hook():
    """Register the axon NTFF profiling hook (the image's antenv lacks
    axon_hooks, so boot skipped registration). Trace-only; best-effort."""
    try:
        import types

        import antenv

        if getattr(antenv, "axon_hooks", None) is not None:
            return
        mod = types.ModuleType("antenv.axon_hooks")
        _h = [None]
        mod.set_axon_ntff_profile_hook = lambda h: _h.__setitem__(0, h)
        mod.get_axon_ntff_profile_hook = lambda: _h[0]
        sys.modules["antenv.axon_hooks"] = mod
        antenv.axon_hooks = mod
        from trn_agent_boot.trn_boot import _ntff_profile_via_ctypes

        hook = _ntff_profile_via_ctypes("/opt/axon/libaxon_pjrt.so")
        if hook is not None:
            mod.set_axon_ntff_profile_hook(hook)
    except Exception as e:  # noqa: BLE001
        print(f"ntff hook install failed (timing unavailable): {e}")


def kernel(x, ln_gamma, ln_beta, W_qkv):
    x = np.asarray(x, dtype=np.float32)
    ln_gamma = np.asarray(ln_gamma, dtype=np.float32)
    ln_beta = np.asarray(ln_beta, dtype=np.float32)
    W_qkv = np.asarray(W_qkv, dtype=np.float32)
    assert x.shape == (B, H, W, C) and W_qkv.shape == (C, F2)

    # fold gamma/beta into the projection (1x1 conv has no bias of its own)
    Wp = (ln_gamma.astype(np.float64)[:, None] * W_qkv.astype(np.float64))
    bW = (ln_beta.astype(np.float64) @ W_qkv.astype(np.float64)).astype(np.float32)
    with_bias = bool(np.any(bW != 0.0))

    key = with_bias
    if key not in _NC_CACHE:
        _NC_CACHE[key] = _build(with_bias)
    nc = _NC_CACHE[key]

    # interleave to [b, s, p, c, t] so bn_stats' even/odd split separates
    # the two w-rows sharing a partition
    x16 = np.ascontiguousarray(
        x.reshape(B, H, 2, 128, C).transpose(0, 1, 3, 4, 2).astype(np.float16))
    wqk = np.ascontiguousarray(
        Wp[:, :256].astype(np.float16).reshape(2, 128, 256))
    wv = np.ascontiguousarray(
        Wp[:, 256:].astype(np.float16).reshape(2, 128, 256))
    in_maps = []
    for b in range(B):
        m = {"x": np.ascontiguousarray(x16[b]), "wqk": wqk, "wv": wv}
        if with_bias:
            m["bqk"] = np.ascontiguousarray(bW[:256].reshape(2, 128))
            m["bv"] = np.ascontiguousarray(bW[256:])
        in_maps.append(m)

    trace = os.environ.get("KERNEL_TRACE", "") == "1"
    if trace:
        _install_ntff_hook()
    res = run_bass_kernel_spmd(nc, in_maps, core_ids=list(range(B)), trace=trace)
    if trace and res.exec_time_ns is not None:
        print(f"HW exec time: {res.exec_time_ns} ns")
        if res.instructions_and_trace is not None:
            print(f"trace: {res.instructions_and_trace[1]}")
    out = np.stack([np.asarray(res.results[b]["out"]) for b in range(B)], axis=0)
    return out.reshape(B, H, W, C).astype(np.float32)
